# revision 27
# baseline (speedup 1.0000x reference)
"""Trainium2 Bass kernel: EdgeModelConcat (GNN edge MLP).

reference math (per edge e):
    x   = concat([dest[e], src[e], u[batch[e]]])      # [192]
    h   = relu(x @ W1 + b1)                            # [256]
    out = h @ W2 + b2                                  # [64]
(edge_attr is an input but unused by the reference.)

Strategy
--------
Data-parallel over edges on 8 NeuronCores.  All layout marshalling is done
on the host so the device only does DMAs + matmuls + fused bias/relu:

* host passes x^T = [dest^T; src^T] as a [128, E/8] f32 array per core, so
  layer-1 is out = W1[:128].T @ x^T with K=128, no on-device transposes.
* the u-term is folded away:  c[g] = u[g] @ W1[128:] + b1  is computed once
  on-device ([256, 512] table).  `batch` is sorted, so per 512-edge tile
  the bias column c[:, g] is piecewise constant; segment boundaries are
  baked into the instruction stream as static column ranges of the fused
  relu+bias ops.  Per-core segment structure differs -> one 8-way
  tc.Switch on partition_id with per-core straight-line code.
* layer-2 keeps the h^T layout; out^T tiles [64, 512] are packed two tiles
  deep into 128 partitions for full-rate DMA stores; the host un-packs.
* software-pipelined emission (layer-2 of tile t-1 issued after layer-1 of
  tile t) keeps the PE from stalling on the ACT/DVE PSUM evacuation.

Matmuls run as float32r (fp32 storage, single-pass PE mode: 1 col/cycle at
N=512, measured end-to-end rel-err ~1.7e-4) or bf16 (half input traffic,
~10% faster, rel-err ~2.8e-3).  MODE="f32r" is the safe default.
"""

import numpy as np

MODE = "f32r"  # "f32r" | "bf16" | "f32"
PROFILE = False            # set True (with NTFF hook installed) to measure
LAST_EXEC_NS = None        # exec time of slowest profiled core, ns
LAST_RESULTS = None

NCORES = 8
TILE = 512                 # edges per matmul tile (PSUM bank = 512 f32)
SLAB_TILES = 8             # tiles per DMA slab (4096 edges = 2MB f32 in)

_cache = {}


def _mode_dtypes():
    import concourse.mybir as mybir

    if MODE == "bf16":
        import ml_dtypes

        return mybir.dt.bfloat16, np.dtype(ml_dtypes.bfloat16), None
    if MODE == "f32r":
        return mybir.dt.float32r, np.dtype(np.float32), None
    return mybir.dt.float32, np.dtype(np.float32), None


def _segments_per_tile(bk, ec, ntiles):
    """bk: per-core sorted graph ids [ec] -> list per tile of (a, b, g)."""
    out = []
    for t in range(ntiles):
        c0 = t * TILE
        w = min(TILE, ec - c0)
        vals = bk[c0 : c0 + w]
        bounds = np.flatnonzero(np.diff(vals)) + 1
        starts = np.concatenate([[0], bounds, [w]])
        out.append(
            [
                (int(starts[i]), int(starts[i + 1]), int(vals[starts[i]]))
                for i in range(len(starts) - 1)
            ]
        )
    return out


def _build(all_segs, ec, fx, fu, h, fo, b, out_w):
    from contextlib import ExitStack

    import concourse.bass as bass
    import concourse.mybir as mybir
    import concourse.tile as tile
    from concourse import bacc

    F32 = mybir.dt.float32
    Relu = mybir.ActivationFunctionType.Relu
    Ident = mybir.ActivationFunctionType.Identity
    ADD = mybir.AluOpType.add
    MAX = mybir.AluOpType.max

    mmdt, _, castdt = _mode_dtypes()

    def mm(ap):
        return ap.bitcast(castdt) if castdt is not None else ap

    ntiles = (ec + TILE - 1) // TILE
    nslabs = (ntiles + SLAB_TILES - 1) // SLAB_TILES
    slab = TILE * SLAB_TILES
    kin = 2 * fx            # 128: contraction dim of layer 1
    mh = h // 128           # 2: H chunks of 128
    assert kin == 128 and h == 256 and fo <= 64

    nc = bacc.Bacc("TRN2", target_bir_lowering=False, debug=False, num_devices=NCORES)
    # packed constants: cf (f32) = [uT | w1u | b1r | b2c] on 128 partitions,
    # cb (matmul dtype) = [w1ds | w2c]
    cf_w = b + h + mh + 1
    cb_w = h + mh * fo
    xT = nc.declare_dram_parameter("xT", [kin, ec], mmdt, isOutput=False)
    cf = nc.declare_dram_parameter("cf", [128, cf_w], F32, isOutput=False)
    cb = nc.declare_dram_parameter("cb", [128, cb_w], mmdt, isOutput=False)
    outT = nc.declare_dram_parameter("outT", [128, out_w], F32, isOutput=True)

    with tile.TileContext(nc) as tc, ExitStack() as ctx:
        pid = nc.partition_id()

        const = ctx.enter_context(tc.tile_pool(name="const", bufs=1))
        xp = ctx.enter_context(tc.tile_pool(name="xp", bufs=3))
        hp = ctx.enter_context(tc.tile_pool(name="hp", bufs=8))
        op = ctx.enter_context(tc.tile_pool(name="op", bufs=3))
        ph0 = ctx.enter_context(tc.tile_pool(name="ph0", bufs=3, space="PSUM"))
        ph1 = ctx.enter_context(tc.tile_pool(name="ph1", bufs=2, space="PSUM"))
        po = ctx.enter_context(tc.tile_pool(name="po", bufs=2, space="PSUM"))

        cf_sb = const.tile([128, cf_w], F32)
        nc.sync.dma_start(cf_sb[:], cf[:])
        cb_sb = const.tile([128, cb_w], mmdt)
        nc.sync.dma_start(cb_sb[:], cb[:])
        uT_sb = cf_sb[0:fu, 0:b]
        w1u_sb = cf_sb[0:fu, b : b + h]
        b1r_sb = cf_sb[:, b + h : b + h + mh]
        b2c_sb = cf_sb[:, b + h + mh : b + h + mh + 1]
        w1ds_sb = cb_sb[:, 0:h]
        w2c_sb = cb_sb[:, h : h + mh * fo]

        # c^T[m][:, g] = (u[g] @ W1[128:192] + b1)[128m : 128m+128], full fp32
        cT_sb = const.tile([128, mh * b], F32)
        for m in range(mh):
            cps = po.tile([128, b], F32, tag="o", name="cps")
            nc.tensor.matmul(
                cps[:], w1u_sb[:, m * 128 : (m + 1) * 128], uT_sb[:],
                start=True, stop=True,
            )
            nc.scalar.activation(
                cT_sb[:, m * b : (m + 1) * b], cps[:], Ident,
                bias=b1r_sb[:, m : m + 1],
            )

        for core in tc.Switch(pid, NCORES):
            segs_per_tile = all_segs[core]
            # software pipeline: at loop index t emit L1(t)+relu(t), then
            # L2(t-1) (whose relu outputs are already in flight), the pair
            # evacuation when t-1 closes a pair, and the slab store when
            # t-1 closes a slab.
            xt = None
            ots = {}       # slab index -> sbuf out tile
            hss = {}       # tile index -> relu'd h tile (sbuf)
            widths = {}
            o_pair = None

            pair_pack = MODE == "bf16"  # fp32-family can't col-offset PSUM

            def store_slab(tp, w, ot):
                # slab store once its last tile's outputs are evacuated
                if tp == ntiles - 1 or tp % SLAB_TILES == SLAB_TILES - 1:
                    nt = (tp % SLAB_TILES) + 1
                    oc0 = (tp // SLAB_TILES) * (slab // 2)
                    npairs = nt // 2
                    if npairs:
                        nc.sync.dma_start(
                            outT[:, oc0 : oc0 + npairs * TILE],
                            ot[:, : npairs * TILE],
                        )
                    if nt % 2:
                        cl = npairs * TILE
                        nc.sync.dma_start(
                            outT[0:fo, oc0 + cl : oc0 + cl + w],
                            ot[0:fo, cl : cl + w],
                        )

            def emit_l2_bf16(tp):
                nonlocal o_pair
                w = widths.pop(tp)
                hst = hss.pop(tp)
                if tp % 2 == 0:
                    o_pair = po.tile([128, TILE], F32, tag="o", name="o_pair")
                r0 = (tp % 2) * 64
                o_dst = o_pair[r0 : r0 + fo, :w]
                nc.tensor.matmul(
                    o_dst, mm(w2c_sb[:, 0:fo]),
                    mm(hst[:, 0:w]), start=True, stop=False,
                )
                nc.tensor.matmul(
                    o_dst, mm(w2c_sb[:, fo : 2 * fo]),
                    mm(hst[:, TILE : TILE + w]), start=False, stop=True,
                )
                s = tp // SLAB_TILES
                cc = ((tp % SLAB_TILES) // 2) * TILE
                ot = ots[s]
                if tp % 2 == 1:
                    # full pair resident: single aligned evacuation
                    if (tp // 2) % 2 == 0:
                        nc.vector.tensor_scalar(
                            out=ot[:, cc : cc + TILE], in0=o_pair[:],
                            scalar1=b2c_sb[:], scalar2=None, op0=ADD,
                        )
                    else:
                        nc.scalar.activation(
                            ot[:, cc : cc + TILE], o_pair[:], Ident,
                            bias=b2c_sb[:],
                        )
                elif tp == ntiles - 1:
                    # trailing unpaired tile: top half only
                    nc.vector.tensor_scalar(
                        out=ot[0:fo, cc : cc + w], in0=o_pair[0:fo, :w],
                        scalar1=b2c_sb[0:fo, :], scalar2=None, op0=ADD,
                    )
                store_slab(tp, w, ot)

            def emit_l2_group(tps):
                if pair_pack:
                    for tp in tps:
                        emit_l2_bf16(tp)
                    return
                # f32r: per-tile PSUM banks -> group same-weight matmuls
                infos = []
                for tp in tps:
                    w = widths.pop(tp)
                    hst = hss.pop(tp)
                    o_t = po.tile([fo, TILE], F32, tag="o", name="o_t")
                    infos.append((tp, w, hst, o_t))
                for (tp, w, hst, o_t) in infos:
                    nc.tensor.matmul(
                        o_t[:, :w], mm(w2c_sb[:, 0:fo]),
                        mm(hst[:, 0:w]), start=True, stop=False,
                    )
                for (tp, w, hst, o_t) in infos:
                    nc.tensor.matmul(
                        o_t[:, :w], mm(w2c_sb[:, fo : 2 * fo]),
                        mm(hst[:, TILE : TILE + w]), start=False, stop=True,
                    )
                for (tp, w, hst, o_t) in infos:
                    s = tp // SLAB_TILES
                    cc = ((tp % SLAB_TILES) // 2) * TILE
                    ot = ots[s]
                    r0 = (tp % 2) * 64
                    if tp % 2 == 0:
                        nc.vector.tensor_scalar(
                            out=ot[r0 : r0 + fo, cc : cc + w], in0=o_t[:, :w],
                            scalar1=b2c_sb[0:fo, :], scalar2=None, op0=ADD,
                        )
                    else:
                        nc.scalar.activation(
                            ot[r0 : r0 + fo, cc : cc + w], o_t[:, :w], Ident,
                            bias=b2c_sb[0:fo, :],
                        )
                    store_slab(tp, w, ot)

            # L1 for a group of tiles, same-weight matmuls adjacent so
            # walrus ldw-opt can elide redundant LDWEIGHTS
            def load_slab(s):
                if s in xts or s >= nslabs:
                    return
                c0 = s * slab
                ws = min(slab, ec - c0)
                xtn = xp.tile([kin, slab], mmdt, tag="xt", name="xt")
                xts[s] = xtn
                nc.sync.dma_start(xtn[:, :ws], xT[:, c0 : c0 + ws])
                ots[s] = op.tile([128, slab // 2], F32, tag="ot", name="ot")

            def emit_l1(group):
                tiles = []
                for t in group:
                    s, j = divmod(t, SLAB_TILES)
                    load_slab(s)
                    if j == 0:
                        # prefetch the following slab a full slab ahead
                        load_slab(s + 1)
                    a = j * TILE
                    w = min(TILE, ec - t * TILE)
                    widths[t] = w
                    h0 = ph0.tile([128, TILE], F32, tag="h0", name="h0")
                    h1 = ph1.tile([128, TILE], F32, tag="h1", name="h1")
                    tiles.append((t, xts[s], a, w, h0, h1))
                for (t, xtt, a, w, h0, h1) in tiles:
                    nc.tensor.matmul(
                        h0[:, :w], mm(w1ds_sb[:, 0:128]), mm(xtt[:, a : a + w]),
                        start=True, stop=True,
                    )
                for (t, xtt, a, w, h0, h1) in tiles:
                    nc.tensor.matmul(
                        h1[:, :w], mm(w1ds_sb[:, 128:256]), mm(xtt[:, a : a + w]),
                        start=True, stop=True,
                    )
                for (t, xtt, a, w, h0, h1) in tiles:
                    hs = hp.tile([128, 2 * TILE], mmdt, tag="hs", name="hs")
                    hss[t] = hs
                    for (sa, sb, g) in segs_per_tile[t]:
                        nc.scalar.activation(
                            hs[:, sa:sb], h0[:, sa:sb], Relu,
                            bias=cT_sb[:, g : g + 1],
                        )
                        nc.vector.tensor_scalar(
                            out=hs[:, TILE + sa : TILE + sb], in0=h1[:, sa:sb],
                            scalar1=cT_sb[:, b + g : b + g + 1], scalar2=0.0,
                            op0=ADD, op1=MAX,
                        )

            xts = {}
            pairs = [
                list(range(p, min(p + 2, ntiles))) for p in range(0, ntiles, 2)
            ]
            for i, group in enumerate(pairs):
                emit_l1(group)
                if i > 0:
                    emit_l2_group(pairs[i - 1])
            emit_l2_group(pairs[-1])
    nc.compile()
    return nc


def kernel(**inputs):
    global LAST_EXEC_NS, LAST_RESULTS

    src = np.asarray(inputs["src"], dtype=np.float32)
    dest = np.asarray(inputs["dest"], dtype=np.float32)
    u = np.asarray(inputs["u"], dtype=np.float32)
    batch = np.asarray(inputs["batch"])
    W1 = np.asarray(inputs["W1"], dtype=np.float32)
    b1 = np.asarray(inputs["b1"], dtype=np.float32)
    W2 = np.asarray(inputs["W2"], dtype=np.float32)
    b2 = np.asarray(inputs["b2"], dtype=np.float32)

    e, fx = src.shape
    b_, fu = u.shape
    h = W1.shape[1]
    fo = W2.shape[1]
    ec = (e + NCORES - 1) // NCORES
    ntiles = (ec + TILE - 1) // TILE

    # sorted edge order (identity when batch already sorted, as speced)
    bi = batch.astype(np.int64)
    if np.any(bi[1:] < bi[:-1]):
        perm = np.argsort(bi, kind="stable")
    else:
        perm = None

    # host-side marshalling ------------------------------------------------
    _, npdt, _ = _mode_dtypes()
    bs = bi if perm is None else bi[perm]
    all_segs = []
    in_maps = []

    # out column layout: tile t -> cols [S(t), S(t)+w) rows (t%2)*64 where
    # S(t) = (t//8)*8*TILE//2 + ((t%8)//2)*TILE
    def out_col(t):
        return (t // SLAB_TILES) * (SLAB_TILES // 2) * TILE + ((t % SLAB_TILES) // 2) * TILE

    wlast = ec - (ntiles - 1) * TILE
    out_w = max(out_col(ntiles - 1) + wlast, out_col(max(ntiles - 2, 0)) + TILE)

    mh = h // 128
    cf = np.zeros((128, b_ + h + mh + 1), dtype=np.float32)
    cf[:fu, :b_] = u.T
    cf[: W1.shape[0] - 2 * fx, b_ : b_ + h] = W1[2 * fx :]
    cf[:, b_ + h : b_ + h + mh] = b1.reshape(mh, 128).T
    cf[:, b_ + h + mh] = np.tile(b2, mh)
    cb = np.concatenate(
        [W1[: 2 * fx]]
        + [np.concatenate([W2[i * 128 : (i + 1) * 128] for i in range(mh)], axis=1)],
        axis=1,
    ).astype(npdt)
    cb = np.ascontiguousarray(cb)

    for k in range(NCORES):
        i0, i1 = k * ec, min((k + 1) * ec, e)
        n = i1 - i0
        if perm is None:
            d_k = dest[i0:i1]
            s_k = src[i0:i1]
        else:
            idx = perm[i0:i1]
            d_k = dest[idx]
            s_k = src[idx]
        xTk = np.empty((2 * fx, ec), dtype=npdt)
        xTk[:fx, :n] = d_k.T
        xTk[fx:, :n] = s_k.T
        if n < ec:
            xTk[:, n:] = 0
        bk = np.empty(ec, dtype=np.int64)
        bk[:n] = bs[i0:i1]
        if n < ec:
            bk[n:] = bk[n - 1]
        all_segs.append(_segments_per_tile(bk, ec, ntiles))
        in_maps.append({"xT": xTk, "cf": cf, "cb": cb})

    # build / fetch compiled program --------------------------------------
    key = (MODE, e, fx, fu, h, fo, b_, hash(bs.tobytes()))
    nc = _cache.get(key)
    if nc is None:
        nc = _build(all_segs, ec, fx, fu, h, fo, b_, out_w)
        _cache.clear()
        _cache[key] = nc

    from concourse.bass_utils import run_bass_kernel_spmd

    res = run_bass_kernel_spmd(
        nc, in_maps, list(range(NCORES)), trace=bool(PROFILE)
    )
    LAST_EXEC_NS = res.exec_time_ns
    LAST_RESULTS = res

    # unpack ---------------------------------------------------------------
    out = np.empty((e, fo), dtype=np.float32)
    for k in range(NCORES):
        o = res.results[k]["outT"]
        i0, i1 = k * ec, min((k + 1) * ec, e)
        n = i1 - i0
        ok = np.empty((ec, fo), dtype=np.float32)
        for t in range(ntiles):
            w = min(TILE, ec - t * TILE)
            c = out_col(t)
            r = (t % 2) * 64
            ok[t * TILE : t * TILE + w] = o[r : r + fo, c : c + w].T
        if perm is None:
            out[i0:i1] = ok[:n]
        else:
            out[perm[i0:i1]] = ok[:n]
    return out


if __name__ == "__main__":
    # small self-test with synthetic inputs (E scaled down)
    rng = np.random.default_rng(0)
    E, FX, FU, H, FO, B = 40960, 64, 64, 256, 64, 512
    src = rng.standard_normal((E, FX), dtype=np.float32)
    dest = rng.standard_normal((E, FX), dtype=np.float32)
    u = rng.standard_normal((B, FU), dtype=np.float32)
    batch = np.sort(rng.integers(0, B, E)).astype(np.int64)
    W1 = (rng.standard_normal((2 * FX + FU, H), dtype=np.float32) / np.sqrt(2 * FX + FU))
    b1 = np.zeros(H, np.float32)
    W2 = rng.standard_normal((H, FO), dtype=np.float32) / np.sqrt(H)
    b2 = np.zeros(FO, np.float32)
    got = kernel(src=src, dest=dest, edge_attr=src, u=u, batch=batch,
                 W1=W1, b1=b1, W2=W2, b2=b2)
    x = np.concatenate([dest, src, u[batch]], axis=1)
    hh = np.maximum(x @ W1 + b1, 0.0)
    want = hh @ W2 + b2
    rel = np.linalg.norm(got - want) / np.linalg.norm(want)
    print("rel err:", rel)


# revision 28
# speedup vs baseline: 1.0604x; 1.0604x over previous
"""Trainium2 Bass kernel: EdgeModelConcat (GNN edge MLP).

reference math (per edge e):
    x   = concat([dest[e], src[e], u[batch[e]]])      # [192]
    h   = relu(x @ W1 + b1)                            # [256]
    out = h @ W2 + b2                                  # [64]
(edge_attr is an input but unused by the reference.)

Strategy
--------
Data-parallel over edges on 8 NeuronCores.  All layout marshalling is done
on the host so the device only does DMAs + matmuls + fused bias/relu:

* host passes x^T = [dest^T; src^T] as a [128, E/8] f32 array per core, so
  layer-1 is out = W1[:128].T @ x^T with K=128, no on-device transposes.
* the u-term is folded away:  c[g] = u[g] @ W1[128:] + b1  is computed once
  on-device ([256, 512] table).  `batch` is sorted, so per 512-edge tile
  the bias column c[:, g] is piecewise constant; segment boundaries are
  baked into the instruction stream as static column ranges of the fused
  relu+bias ops.  Per-core segment structure differs -> one 8-way
  tc.Switch on partition_id with per-core straight-line code.
* layer-2 keeps the h^T layout; out^T tiles [64, 512] are packed two tiles
  deep into 128 partitions for full-rate DMA stores; the host un-packs.
* software-pipelined emission (layer-2 of tile t-1 issued after layer-1 of
  tile t) keeps the PE from stalling on the ACT/DVE PSUM evacuation.

Matmuls run as float32r (fp32 storage, single-pass PE mode: 1 col/cycle at
N=512, measured end-to-end rel-err ~1.7e-4) or bf16 (half input traffic,
~10% faster, rel-err ~2.8e-3).  MODE="f32r" is the safe default.
"""

import numpy as np

MODE = "f32r"  # "f32r" | "bf16" | "f32"
PROFILE = False            # set True (with NTFF hook installed) to measure
LAST_EXEC_NS = None        # exec time of slowest profiled core, ns
LAST_RESULTS = None

NCORES = 8
TILE = 512                 # edges per matmul tile (PSUM bank = 512 f32)
SLAB_TILES = 8             # tiles per DMA slab (4096 edges = 2MB f32 in)

_cache = {}


def _mode_dtypes():
    import concourse.mybir as mybir

    if MODE == "bf16":
        import ml_dtypes

        return mybir.dt.bfloat16, np.dtype(ml_dtypes.bfloat16), None
    if MODE == "f32r":
        return mybir.dt.float32r, np.dtype(np.float32), None
    return mybir.dt.float32, np.dtype(np.float32), None


def _segments_per_tile(bk, ec, ntiles):
    """bk: per-core sorted graph ids [ec] -> list per tile of (a, b, g)."""
    out = []
    for t in range(ntiles):
        c0 = t * TILE
        w = min(TILE, ec - c0)
        vals = bk[c0 : c0 + w]
        bounds = np.flatnonzero(np.diff(vals)) + 1
        starts = np.concatenate([[0], bounds, [w]])
        out.append(
            [
                (int(starts[i]), int(starts[i + 1]), int(vals[starts[i]]))
                for i in range(len(starts) - 1)
            ]
        )
    return out


def _build(all_segs, ec, fx, fu, h, fo, b, out_w):
    from contextlib import ExitStack

    import concourse.bass as bass
    import concourse.mybir as mybir
    import concourse.tile as tile
    from concourse import bacc

    F32 = mybir.dt.float32
    Relu = mybir.ActivationFunctionType.Relu
    Ident = mybir.ActivationFunctionType.Identity
    ADD = mybir.AluOpType.add
    MAX = mybir.AluOpType.max

    mmdt, _, castdt = _mode_dtypes()

    def mm(ap):
        return ap.bitcast(castdt) if castdt is not None else ap

    ntiles = (ec + TILE - 1) // TILE
    nslabs = (ntiles + SLAB_TILES - 1) // SLAB_TILES
    slab = TILE * SLAB_TILES
    kin = 2 * fx            # 128: contraction dim of layer 1
    mh = h // 128           # 2: H chunks of 128
    assert kin == 128 and h == 256 and fo <= 64

    nc = bacc.Bacc("TRN2", target_bir_lowering=False, debug=False, num_devices=NCORES)
    # packed constants: cf (f32) = [uT | w1u | b1r | b2c] on 128 partitions,
    # cb (matmul dtype) = [w1ds | w2c]
    cf_w = b + h + mh + 1
    cb_w = h + mh * fo
    xT = nc.declare_dram_parameter("xT", [kin, ec], mmdt, isOutput=False)
    cf = nc.declare_dram_parameter("cf", [128, cf_w], F32, isOutput=False)
    cb = nc.declare_dram_parameter("cb", [128, cb_w], mmdt, isOutput=False)
    outT = nc.declare_dram_parameter("outT", [128, out_w], F32, isOutput=True)

    with tile.TileContext(nc) as tc, ExitStack() as ctx:
        pid = nc.partition_id()

        const = ctx.enter_context(tc.tile_pool(name="const", bufs=1))
        xp = ctx.enter_context(tc.tile_pool(name="xp", bufs=3))
        hp = ctx.enter_context(tc.tile_pool(name="hp", bufs=8))
        op = ctx.enter_context(tc.tile_pool(name="op", bufs=3))
        ph0 = ctx.enter_context(tc.tile_pool(name="ph0", bufs=3, space="PSUM"))
        ph1 = ctx.enter_context(tc.tile_pool(name="ph1", bufs=2, space="PSUM"))
        po = ctx.enter_context(tc.tile_pool(name="po", bufs=2, space="PSUM"))

        cf_sb = const.tile([128, cf_w], F32)
        nc.sync.dma_start(cf_sb[:], cf[:])
        cb_sb = const.tile([128, cb_w], mmdt)
        nc.sync.dma_start(cb_sb[:], cb[:])
        uT_sb = cf_sb[0:fu, 0:b]
        w1u_sb = cf_sb[0:fu, b : b + h]
        b1r_sb = cf_sb[:, b + h : b + h + mh]
        b2c_sb = cf_sb[:, b + h + mh : b + h + mh + 1]
        w1ds_sb = cb_sb[:, 0:h]
        w2c_sb = cb_sb[:, h : h + mh * fo]

        # c^T[m][:, g] = (u[g] @ W1[128:192] + b1)[128m : 128m+128], full fp32
        cT_sb = const.tile([128, mh * b], F32)
        for m in range(mh):
            cps = po.tile([128, b], F32, tag="o", name="cps")
            nc.tensor.matmul(
                cps[:], w1u_sb[:, m * 128 : (m + 1) * 128], uT_sb[:],
                start=True, stop=True,
            )
            nc.scalar.activation(
                cT_sb[:, m * b : (m + 1) * b], cps[:], Ident,
                bias=b1r_sb[:, m : m + 1],
            )

        for core in tc.Switch(pid, NCORES):
            segs_per_tile = all_segs[core]
            # software pipeline: at loop index t emit L1(t)+relu(t), then
            # L2(t-1) (whose relu outputs are already in flight), the pair
            # evacuation when t-1 closes a pair, and the slab store when
            # t-1 closes a slab.
            xt = None
            ots = {}       # slab index -> sbuf out tile
            hss = {}       # tile index -> relu'd h tile (sbuf)
            widths = {}
            o_pair = None

            pair_pack = MODE == "bf16"  # fp32-family can't col-offset PSUM

            def store_slab(tp, w, ot):
                # slab store once its last tile's outputs are evacuated
                if tp == ntiles - 1 or tp % SLAB_TILES == SLAB_TILES - 1:
                    nt = (tp % SLAB_TILES) + 1
                    oc0 = (tp // SLAB_TILES) * (slab // 2)
                    npairs = nt // 2
                    if npairs:
                        nc.sync.dma_start(
                            outT[:, oc0 : oc0 + npairs * TILE],
                            ot[:, : npairs * TILE],
                        )
                    if nt % 2:
                        cl = npairs * TILE
                        nc.sync.dma_start(
                            outT[0:fo, oc0 + cl : oc0 + cl + w],
                            ot[0:fo, cl : cl + w],
                        )

            def emit_l2_bf16(tp):
                nonlocal o_pair
                w = widths.pop(tp)
                hst = hss.pop(tp)
                if tp % 2 == 0:
                    o_pair = po.tile([128, TILE], F32, tag="o", name="o_pair")
                r0 = (tp % 2) * 64
                o_dst = o_pair[r0 : r0 + fo, :w]
                nc.tensor.matmul(
                    o_dst, mm(w2c_sb[:, 0:fo]),
                    mm(hst[:, 0:w]), start=True, stop=False,
                )
                nc.tensor.matmul(
                    o_dst, mm(w2c_sb[:, fo : 2 * fo]),
                    mm(hst[:, TILE : TILE + w]), start=False, stop=True,
                )
                s = tp // SLAB_TILES
                cc = ((tp % SLAB_TILES) // 2) * TILE
                ot = ots[s]
                if tp % 2 == 1:
                    # full pair resident: single aligned evacuation
                    if (tp // 2) % 2 == 0:
                        nc.vector.tensor_scalar(
                            out=ot[:, cc : cc + TILE], in0=o_pair[:],
                            scalar1=b2c_sb[:], scalar2=None, op0=ADD,
                        )
                    else:
                        nc.scalar.activation(
                            ot[:, cc : cc + TILE], o_pair[:], Ident,
                            bias=b2c_sb[:],
                        )
                elif tp == ntiles - 1:
                    # trailing unpaired tile: top half only
                    nc.vector.tensor_scalar(
                        out=ot[0:fo, cc : cc + w], in0=o_pair[0:fo, :w],
                        scalar1=b2c_sb[0:fo, :], scalar2=None, op0=ADD,
                    )
                store_slab(tp, w, ot)

            def emit_l2_group(tps):
                if pair_pack:
                    for tp in tps:
                        emit_l2_bf16(tp)
                    return
                # f32r: per-tile PSUM banks, per-tile order (grouping the
                # same-weight matmuls delays evacuation and measures slower)
                for tp in tps:
                    w = widths.pop(tp)
                    hst = hss.pop(tp)
                    o_t = po.tile([fo, TILE], F32, tag="o", name="o_t")
                    nc.tensor.matmul(
                        o_t[:, :w], mm(w2c_sb[:, 0:fo]),
                        mm(hst[:, 0:w]), start=True, stop=False,
                    )
                    nc.tensor.matmul(
                        o_t[:, :w], mm(w2c_sb[:, fo : 2 * fo]),
                        mm(hst[:, TILE : TILE + w]), start=False, stop=True,
                    )
                    s = tp // SLAB_TILES
                    cc = ((tp % SLAB_TILES) // 2) * TILE
                    ot = ots[s]
                    r0 = (tp % 2) * 64
                    if tp % 2 == 0:
                        nc.vector.tensor_scalar(
                            out=ot[r0 : r0 + fo, cc : cc + w], in0=o_t[:, :w],
                            scalar1=b2c_sb[0:fo, :], scalar2=None, op0=ADD,
                        )
                    else:
                        nc.scalar.activation(
                            ot[r0 : r0 + fo, cc : cc + w], o_t[:, :w], Ident,
                            bias=b2c_sb[0:fo, :],
                        )
                    store_slab(tp, w, ot)

            # L1 for a group of tiles, same-weight matmuls adjacent so
            # walrus ldw-opt can elide redundant LDWEIGHTS
            def load_slab(s):
                if s in xts or s >= nslabs:
                    return
                c0 = s * slab
                ws = min(slab, ec - c0)
                xtn = xp.tile([kin, slab], mmdt, tag="xt", name="xt")
                xts[s] = xtn
                nc.sync.dma_start(xtn[:, :ws], xT[:, c0 : c0 + ws])
                ots[s] = op.tile([128, slab // 2], F32, tag="ot", name="ot")

            def emit_l1(group):
                tiles = []
                for t in group:
                    s, j = divmod(t, SLAB_TILES)
                    load_slab(s)
                    if j == 0:
                        # prefetch the following slab a full slab ahead
                        load_slab(s + 1)
                    a = j * TILE
                    w = min(TILE, ec - t * TILE)
                    widths[t] = w
                    h0 = ph0.tile([128, TILE], F32, tag="h0", name="h0")
                    h1 = ph1.tile([128, TILE], F32, tag="h1", name="h1")
                    tiles.append((t, xts[s], a, w, h0, h1))
                for (t, xtt, a, w, h0, h1) in tiles:
                    nc.tensor.matmul(
                        h0[:, :w], mm(w1ds_sb[:, 0:128]), mm(xtt[:, a : a + w]),
                        start=True, stop=True,
                    )
                for (t, xtt, a, w, h0, h1) in tiles:
                    nc.tensor.matmul(
                        h1[:, :w], mm(w1ds_sb[:, 128:256]), mm(xtt[:, a : a + w]),
                        start=True, stop=True,
                    )
                for (t, xtt, a, w, h0, h1) in tiles:
                    hs = hp.tile([128, 2 * TILE], mmdt, tag="hs", name="hs")
                    hss[t] = hs
                    for (sa, sb, g) in segs_per_tile[t]:
                        nc.scalar.activation(
                            hs[:, sa:sb], h0[:, sa:sb], Relu,
                            bias=cT_sb[:, g : g + 1],
                        )
                        nc.vector.tensor_scalar(
                            out=hs[:, TILE + sa : TILE + sb], in0=h1[:, sa:sb],
                            scalar1=cT_sb[:, b + g : b + g + 1], scalar2=0.0,
                            op0=ADD, op1=MAX,
                        )

            xts = {}
            pairs = [
                list(range(p, min(p + 2, ntiles))) for p in range(0, ntiles, 2)
            ]
            for i, group in enumerate(pairs):
                emit_l1(group)
                if i > 0:
                    emit_l2_group(pairs[i - 1])
            emit_l2_group(pairs[-1])
    nc.compile()
    return nc


def kernel(**inputs):
    global LAST_EXEC_NS, LAST_RESULTS

    src = np.asarray(inputs["src"], dtype=np.float32)
    dest = np.asarray(inputs["dest"], dtype=np.float32)
    u = np.asarray(inputs["u"], dtype=np.float32)
    batch = np.asarray(inputs["batch"])
    W1 = np.asarray(inputs["W1"], dtype=np.float32)
    b1 = np.asarray(inputs["b1"], dtype=np.float32)
    W2 = np.asarray(inputs["W2"], dtype=np.float32)
    b2 = np.asarray(inputs["b2"], dtype=np.float32)

    e, fx = src.shape
    b_, fu = u.shape
    h = W1.shape[1]
    fo = W2.shape[1]
    ec = (e + NCORES - 1) // NCORES
    ntiles = (ec + TILE - 1) // TILE

    # sorted edge order (identity when batch already sorted, as speced)
    bi = batch.astype(np.int64)
    if np.any(bi[1:] < bi[:-1]):
        perm = np.argsort(bi, kind="stable")
    else:
        perm = None

    # host-side marshalling ------------------------------------------------
    _, npdt, _ = _mode_dtypes()
    bs = bi if perm is None else bi[perm]
    all_segs = []
    in_maps = []

    # out column layout: tile t -> cols [S(t), S(t)+w) rows (t%2)*64 where
    # S(t) = (t//8)*8*TILE//2 + ((t%8)//2)*TILE
    def out_col(t):
        return (t // SLAB_TILES) * (SLAB_TILES // 2) * TILE + ((t % SLAB_TILES) // 2) * TILE

    wlast = ec - (ntiles - 1) * TILE
    out_w = max(out_col(ntiles - 1) + wlast, out_col(max(ntiles - 2, 0)) + TILE)

    mh = h // 128
    cf = np.zeros((128, b_ + h + mh + 1), dtype=np.float32)
    cf[:fu, :b_] = u.T
    cf[: W1.shape[0] - 2 * fx, b_ : b_ + h] = W1[2 * fx :]
    cf[:, b_ + h : b_ + h + mh] = b1.reshape(mh, 128).T
    cf[:, b_ + h + mh] = np.tile(b2, mh)
    cb = np.concatenate(
        [W1[: 2 * fx]]
        + [np.concatenate([W2[i * 128 : (i + 1) * 128] for i in range(mh)], axis=1)],
        axis=1,
    ).astype(npdt)
    cb = np.ascontiguousarray(cb)

    for k in range(NCORES):
        i0, i1 = k * ec, min((k + 1) * ec, e)
        n = i1 - i0
        if perm is None:
            d_k = dest[i0:i1]
            s_k = src[i0:i1]
        else:
            idx = perm[i0:i1]
            d_k = dest[idx]
            s_k = src[idx]
        xTk = np.empty((2 * fx, ec), dtype=npdt)
        xTk[:fx, :n] = d_k.T
        xTk[fx:, :n] = s_k.T
        if n < ec:
            xTk[:, n:] = 0
        bk = np.empty(ec, dtype=np.int64)
        bk[:n] = bs[i0:i1]
        if n < ec:
            bk[n:] = bk[n - 1]
        all_segs.append(_segments_per_tile(bk, ec, ntiles))
        in_maps.append({"xT": xTk, "cf": cf, "cb": cb})

    # build / fetch compiled program --------------------------------------
    key = (MODE, e, fx, fu, h, fo, b_, hash(bs.tobytes()))
    nc = _cache.get(key)
    if nc is None:
        nc = _build(all_segs, ec, fx, fu, h, fo, b_, out_w)
        _cache.clear()
        _cache[key] = nc

    from concourse.bass_utils import run_bass_kernel_spmd

    res = run_bass_kernel_spmd(
        nc, in_maps, list(range(NCORES)), trace=bool(PROFILE)
    )
    LAST_EXEC_NS = res.exec_time_ns
    LAST_RESULTS = res

    # unpack ---------------------------------------------------------------
    out = np.empty((e, fo), dtype=np.float32)
    for k in range(NCORES):
        o = res.results[k]["outT"]
        i0, i1 = k * ec, min((k + 1) * ec, e)
        n = i1 - i0
        ok = np.empty((ec, fo), dtype=np.float32)
        for t in range(ntiles):
            w = min(TILE, ec - t * TILE)
            c = out_col(t)
            r = (t % 2) * 64
            ok[t * TILE : t * TILE + w] = o[r : r + fo, c : c + w].T
        if perm is None:
            out[i0:i1] = ok[:n]
        else:
            out[perm[i0:i1]] = ok[:n]
    return out


if __name__ == "__main__":
    # small self-test with synthetic inputs (E scaled down)
    rng = np.random.default_rng(0)
    E, FX, FU, H, FO, B = 40960, 64, 64, 256, 64, 512
    src = rng.standard_normal((E, FX), dtype=np.float32)
    dest = rng.standard_normal((E, FX), dtype=np.float32)
    u = rng.standard_normal((B, FU), dtype=np.float32)
    batch = np.sort(rng.integers(0, B, E)).astype(np.int64)
    W1 = (rng.standard_normal((2 * FX + FU, H), dtype=np.float32) / np.sqrt(2 * FX + FU))
    b1 = np.zeros(H, np.float32)
    W2 = rng.standard_normal((H, FO), dtype=np.float32) / np.sqrt(H)
    b2 = np.zeros(FO, np.float32)
    got = kernel(src=src, dest=dest, edge_attr=src, u=u, batch=batch,
                 W1=W1, b1=b1, W2=W2, b2=b2)
    x = np.concatenate([dest, src, u[batch]], axis=1)
    hh = np.maximum(x @ W1 + b1, 0.0)
    want = hh @ W2 + b2
    rel = np.linalg.norm(got - want) / np.linalg.norm(want)
    print("rel err:", rel)


# revision 30
# speedup vs baseline: 1.0627x; 1.0022x over previous
"""Trainium2 Bass kernel: EdgeModelConcat (GNN edge MLP).

reference math (per edge e):
    x   = concat([dest[e], src[e], u[batch[e]]])      # [192]
    h   = relu(x @ W1 + b1)                            # [256]
    out = h @ W2 + b2                                  # [64]
(edge_attr is an input but unused by the reference.)

Strategy
--------
Data-parallel over edges on 8 NeuronCores.  All layout marshalling is done
on the host so the device only does DMAs + matmuls + fused bias/relu:

* host passes x^T = [dest^T; src^T] as a [128, E/8] f32 array per core, so
  layer-1 is out = W1[:128].T @ x^T with K=128, no on-device transposes.
* the u-term is folded away:  c[g] = u[g] @ W1[128:] + b1  is computed once
  on-device ([256, 512] table).  `batch` is sorted, so per 512-edge tile
  the bias column c[:, g] is piecewise constant; segment boundaries are
  baked into the instruction stream as static column ranges of the fused
  relu+bias ops.  Per-core segment structure differs -> one 8-way
  tc.Switch on partition_id with per-core straight-line code.
* layer-2 keeps the h^T layout; out^T tiles [64, 512] are packed two tiles
  deep into 128 partitions for full-rate DMA stores; the host un-packs.
* software-pipelined emission (layer-2 of tile t-1 issued after layer-1 of
  tile t) keeps the PE from stalling on the ACT/DVE PSUM evacuation.

Matmuls run as float32r (fp32 storage, single-pass PE mode: 1 col/cycle at
N=512, measured end-to-end rel-err ~1.7e-4) or bf16 (half input traffic,
~10% faster, rel-err ~2.8e-3).  MODE="f32r" is the safe default.
"""

import numpy as np

MODE = "f32r"  # "f32r" | "bf16" | "f32"
PROFILE = False            # set True (with NTFF hook installed) to measure
LAST_EXEC_NS = None        # exec time of slowest profiled core, ns
LAST_RESULTS = None

NCORES = 8
TILE = 512                 # edges per matmul tile (PSUM bank = 512 f32)
SLAB_TILES = 8             # tiles per DMA slab (4096 edges = 2MB f32 in)

_cache = {}


def _mode_dtypes():
    import concourse.mybir as mybir

    if MODE == "bf16":
        import ml_dtypes

        return mybir.dt.bfloat16, np.dtype(ml_dtypes.bfloat16), None
    if MODE == "f32r":
        return mybir.dt.float32r, np.dtype(np.float32), None
    return mybir.dt.float32, np.dtype(np.float32), None


def _segments_per_tile(bk, ec, ntiles):
    """bk: per-core sorted graph ids [ec] -> list per tile of (a, b, g)."""
    out = []
    for t in range(ntiles):
        c0 = t * TILE
        w = min(TILE, ec - c0)
        vals = bk[c0 : c0 + w]
        bounds = np.flatnonzero(np.diff(vals)) + 1
        starts = np.concatenate([[0], bounds, [w]])
        out.append(
            [
                (int(starts[i]), int(starts[i + 1]), int(vals[starts[i]]))
                for i in range(len(starts) - 1)
            ]
        )
    return out


def _build(all_segs, ec, fx, fu, h, fo, b, out_w):
    from contextlib import ExitStack

    import concourse.bass as bass
    import concourse.mybir as mybir
    import concourse.tile as tile
    from concourse import bacc

    F32 = mybir.dt.float32
    Relu = mybir.ActivationFunctionType.Relu
    Ident = mybir.ActivationFunctionType.Identity
    ADD = mybir.AluOpType.add
    MAX = mybir.AluOpType.max

    mmdt, _, castdt = _mode_dtypes()

    def mm(ap):
        return ap.bitcast(castdt) if castdt is not None else ap

    ntiles = (ec + TILE - 1) // TILE
    nslabs = (ntiles + SLAB_TILES - 1) // SLAB_TILES
    slab = TILE * SLAB_TILES
    kin = 2 * fx            # 128: contraction dim of layer 1
    mh = h // 128           # 2: H chunks of 128
    assert kin == 128 and h == 256 and fo <= 64

    nc = bacc.Bacc("TRN2", target_bir_lowering=False, debug=False, num_devices=NCORES)
    # packed constants: cf (f32) = [uT | w1u | b1r | b2c] on 128 partitions,
    # cb (matmul dtype) = [w1ds | w2c]
    cf_w = b + h + mh + 1
    cb_w = h + mh * fo
    xT = nc.declare_dram_parameter("xT", [kin, ec], mmdt, isOutput=False)
    cf = nc.declare_dram_parameter("cf", [128, cf_w], F32, isOutput=False)
    cb = nc.declare_dram_parameter("cb", [128, cb_w], mmdt, isOutput=False)
    outT = nc.declare_dram_parameter("outT", [128, out_w], F32, isOutput=True)

    with tile.TileContext(nc) as tc, ExitStack() as ctx:
        pid = nc.partition_id()

        const = ctx.enter_context(tc.tile_pool(name="const", bufs=1))
        xp = ctx.enter_context(tc.tile_pool(name="xp", bufs=3))
        hp = ctx.enter_context(tc.tile_pool(name="hp", bufs=8))
        op = ctx.enter_context(tc.tile_pool(name="op", bufs=3))
        ph0 = ctx.enter_context(tc.tile_pool(name="ph0", bufs=3, space="PSUM"))
        ph1 = ctx.enter_context(tc.tile_pool(name="ph1", bufs=2, space="PSUM"))
        po = ctx.enter_context(tc.tile_pool(name="po", bufs=2, space="PSUM"))

        cf_sb = const.tile([128, cf_w], F32)
        nc.sync.dma_start(cf_sb[:], cf[:])
        cb_sb = const.tile([128, cb_w], mmdt)
        nc.sync.dma_start(cb_sb[:], cb[:])
        uT_sb = cf_sb[0:fu, 0:b]
        w1u_sb = cf_sb[0:fu, b : b + h]
        b1r_sb = cf_sb[:, b + h : b + h + mh]
        b2c_sb = cf_sb[:, b + h + mh : b + h + mh + 1]
        w1ds_sb = cb_sb[:, 0:h]
        w2c_sb = cb_sb[:, h : h + mh * fo]

        # c^T[m][:, g] = (u[g] @ W1[128:192] + b1)[128m : 128m+128], full fp32
        cT_sb = const.tile([128, mh * b], F32)
        for m in range(mh):
            cps = po.tile([128, b], F32, tag="o", name="cps")
            nc.tensor.matmul(
                cps[:], w1u_sb[:, m * 128 : (m + 1) * 128], uT_sb[:],
                start=True, stop=True,
            )
            nc.scalar.activation(
                cT_sb[:, m * b : (m + 1) * b], cps[:], Ident,
                bias=b1r_sb[:, m : m + 1],
            )

        for core in tc.Switch(pid, NCORES):
            segs_per_tile = all_segs[core]
            # software pipeline: at loop index t emit L1(t)+relu(t), then
            # L2(t-1) (whose relu outputs are already in flight), the pair
            # evacuation when t-1 closes a pair, and the slab store when
            # t-1 closes a slab.
            xt = None
            ots = {}       # slab index -> sbuf out tile
            hss = {}       # tile index -> relu'd h tile (sbuf)
            widths = {}
            o_pair = None

            pair_pack = MODE == "bf16"  # fp32-family can't col-offset PSUM

            def store_slab(tp, w, ot):
                # slab store once its last tile's outputs are evacuated
                if tp == ntiles - 1 or tp % SLAB_TILES == SLAB_TILES - 1:
                    nt = (tp % SLAB_TILES) + 1
                    oc0 = (tp // SLAB_TILES) * (slab // 2)
                    npairs = nt // 2
                    if npairs:
                        nc.sync.dma_start(
                            outT[:, oc0 : oc0 + npairs * TILE],
                            ot[:, : npairs * TILE],
                        )
                    if nt % 2:
                        cl = npairs * TILE
                        nc.sync.dma_start(
                            outT[0:fo, oc0 + cl : oc0 + cl + w],
                            ot[0:fo, cl : cl + w],
                        )

            def emit_l2_bf16(tp):
                nonlocal o_pair
                w = widths.pop(tp)
                hst = hss.pop(tp)
                if tp % 2 == 0:
                    o_pair = po.tile([128, TILE], F32, tag="o", name="o_pair")
                r0 = (tp % 2) * 64
                o_dst = o_pair[r0 : r0 + fo, :w]
                nc.tensor.matmul(
                    o_dst, mm(w2c_sb[:, 0:fo]),
                    mm(hst[:, 0:w]), start=True, stop=False,
                )
                nc.tensor.matmul(
                    o_dst, mm(w2c_sb[:, fo : 2 * fo]),
                    mm(hst[:, TILE : TILE + w]), start=False, stop=True,
                )
                s = tp // SLAB_TILES
                cc = ((tp % SLAB_TILES) // 2) * TILE
                ot = ots[s]
                if tp % 2 == 1:
                    # full pair resident: single aligned evacuation
                    if (tp // 2) % 2 == 0:
                        nc.vector.tensor_scalar(
                            out=ot[:, cc : cc + TILE], in0=o_pair[:],
                            scalar1=b2c_sb[:], scalar2=None, op0=ADD,
                        )
                    else:
                        nc.scalar.activation(
                            ot[:, cc : cc + TILE], o_pair[:], Ident,
                            bias=b2c_sb[:],
                        )
                elif tp == ntiles - 1:
                    # trailing unpaired tile: top half only
                    nc.vector.tensor_scalar(
                        out=ot[0:fo, cc : cc + w], in0=o_pair[0:fo, :w],
                        scalar1=b2c_sb[0:fo, :], scalar2=None, op0=ADD,
                    )
                store_slab(tp, w, ot)

            def emit_l2_group(tps):
                if pair_pack:
                    for tp in tps:
                        emit_l2_bf16(tp)
                    return
                # f32r: per-tile PSUM banks, per-tile order (grouping the
                # same-weight matmuls delays evacuation and measures slower)
                for tp in tps:
                    w = widths.pop(tp)
                    hst = hss.pop(tp)
                    o_t = po.tile([fo, TILE], F32, tag="o", name="o_t")
                    nc.tensor.matmul(
                        o_t[:, :w], mm(w2c_sb[:, 0:fo]),
                        mm(hst[:, 0:w]), start=True, stop=False,
                    )
                    nc.tensor.matmul(
                        o_t[:, :w], mm(w2c_sb[:, fo : 2 * fo]),
                        mm(hst[:, TILE : TILE + w]), start=False, stop=True,
                    )
                    s = tp // SLAB_TILES
                    cc = ((tp % SLAB_TILES) // 2) * TILE
                    ot = ots[s]
                    r0 = (tp % 2) * 64
                    if tp % 2 == 0:
                        nc.vector.tensor_scalar(
                            out=ot[r0 : r0 + fo, cc : cc + w], in0=o_t[:, :w],
                            scalar1=b2c_sb[0:fo, :], scalar2=None, op0=ADD,
                        )
                    else:
                        nc.scalar.activation(
                            ot[r0 : r0 + fo, cc : cc + w], o_t[:, :w], Ident,
                            bias=b2c_sb[0:fo, :],
                        )
                    store_slab(tp, w, ot)

            # L1 for a group of tiles, same-weight matmuls adjacent so
            # walrus ldw-opt can elide redundant LDWEIGHTS
            def load_slab(s):
                if s in xts or s >= nslabs:
                    return
                c0 = s * slab
                ws = min(slab, ec - c0)
                xtn = xp.tile([kin, slab], mmdt, tag="xt", name="xt")
                xts[s] = xtn
                nc.sync.dma_start(xtn[:, :ws], xT[:, c0 : c0 + ws])
                ots[s] = op.tile([128, slab // 2], F32, tag="ot", name="ot")

            def emit_l1(group):
                tiles = []
                for t in group:
                    s, j = divmod(t, SLAB_TILES)
                    load_slab(s)
                    if j == 0:
                        # prefetch the following slab a full slab ahead
                        load_slab(s + 1)
                    a = j * TILE
                    w = min(TILE, ec - t * TILE)
                    widths[t] = w
                    h0 = ph0.tile([128, TILE], F32, tag="h0", name="h0")
                    h1 = ph1.tile([128, TILE], F32, tag="h1", name="h1")
                    tiles.append((t, xts[s], a, w, h0, h1))
                for (t, xtt, a, w, h0, h1) in tiles:
                    nc.tensor.matmul(
                        h0[:, :w], mm(w1ds_sb[:, 0:128]), mm(xtt[:, a : a + w]),
                        start=True, stop=True,
                    )
                for (t, xtt, a, w, h0, h1) in tiles:
                    nc.tensor.matmul(
                        h1[:, :w], mm(w1ds_sb[:, 128:256]), mm(xtt[:, a : a + w]),
                        start=True, stop=True,
                    )
                for (t, xtt, a, w, h0, h1) in tiles:
                    hs = hp.tile([128, 2 * TILE], mmdt, tag="hs", name="hs")
                    hss[t] = hs
                    for (sa, sb, g) in segs_per_tile[t]:
                        nc.scalar.activation(
                            hs[:, sa:sb], h0[:, sa:sb], Relu,
                            bias=cT_sb[:, g : g + 1],
                        )
                        nc.vector.tensor_scalar(
                            out=hs[:, TILE + sa : TILE + sb], in0=h1[:, sa:sb],
                            scalar1=cT_sb[:, b + g : b + g + 1], scalar2=0.0,
                            op0=ADD, op1=MAX,
                        )

            xts = {}
            pairs = [
                list(range(p, min(p + 2, ntiles))) for p in range(0, ntiles, 2)
            ]
            for i, group in enumerate(pairs):
                emit_l1(group)
                if i > 0:
                    emit_l2_group(pairs[i - 1])
            emit_l2_group(pairs[-1])
    nc.compile()
    return nc


def kernel(**inputs):
    global LAST_EXEC_NS, LAST_RESULTS

    src = np.asarray(inputs["src"], dtype=np.float32)
    dest = np.asarray(inputs["dest"], dtype=np.float32)
    u = np.asarray(inputs["u"], dtype=np.float32)
    batch = np.asarray(inputs["batch"])
    W1 = np.asarray(inputs["W1"], dtype=np.float32)
    b1 = np.asarray(inputs["b1"], dtype=np.float32)
    W2 = np.asarray(inputs["W2"], dtype=np.float32)
    b2 = np.asarray(inputs["b2"], dtype=np.float32)

    e, fx = src.shape
    b_, fu = u.shape
    h = W1.shape[1]
    fo = W2.shape[1]
    ec = (e + NCORES - 1) // NCORES
    ntiles = (ec + TILE - 1) // TILE

    # sorted edge order (identity when batch already sorted, as speced)
    bi = batch.astype(np.int64)
    if np.any(bi[1:] < bi[:-1]):
        perm = np.argsort(bi, kind="stable")
    else:
        perm = None

    # host-side marshalling ------------------------------------------------
    _, npdt, _ = _mode_dtypes()
    bs = bi if perm is None else bi[perm]
    all_segs = []
    in_maps = []

    # out column layout: tile t -> cols [S(t), S(t)+w) rows (t%2)*64 where
    # S(t) = (t//8)*8*TILE//2 + ((t%8)//2)*TILE
    def out_col(t):
        return (t // SLAB_TILES) * (SLAB_TILES // 2) * TILE + ((t % SLAB_TILES) // 2) * TILE

    wlast = ec - (ntiles - 1) * TILE
    out_w = max(out_col(ntiles - 1) + wlast, out_col(max(ntiles - 2, 0)) + TILE)

    mh = h // 128
    cf = np.zeros((128, b_ + h + mh + 1), dtype=np.float32)
    cf[:fu, :b_] = u.T
    cf[: W1.shape[0] - 2 * fx, b_ : b_ + h] = W1[2 * fx :]
    cf[:, b_ + h : b_ + h + mh] = b1.reshape(mh, 128).T
    cf[:, b_ + h + mh] = np.tile(b2, mh)
    cb = np.concatenate(
        [W1[: 2 * fx]]
        + [np.concatenate([W2[i * 128 : (i + 1) * 128] for i in range(mh)], axis=1)],
        axis=1,
    ).astype(npdt)
    cb = np.ascontiguousarray(cb)

    for k in range(NCORES):
        i0, i1 = k * ec, min((k + 1) * ec, e)
        n = i1 - i0
        if perm is None:
            d_k = dest[i0:i1]
            s_k = src[i0:i1]
        else:
            idx = perm[i0:i1]
            d_k = dest[idx]
            s_k = src[idx]
        xTk = np.empty((2 * fx, ec), dtype=npdt)
        xTk[:fx, :n] = d_k.T
        xTk[fx:, :n] = s_k.T
        if n < ec:
            xTk[:, n:] = 0
        bk = np.empty(ec, dtype=np.int64)
        bk[:n] = bs[i0:i1]
        if n < ec:
            bk[n:] = bk[n - 1]
        all_segs.append(_segments_per_tile(bk, ec, ntiles))
        in_maps.append({"xT": xTk, "cf": cf, "cb": cb})

    # build / fetch compiled program --------------------------------------
    key = (MODE, e, fx, fu, h, fo, b_, hash(bs.tobytes()))
    nc = _cache.get(key)
    if nc is None:
        nc = _build(all_segs, ec, fx, fu, h, fo, b_, out_w)
        _cache.clear()
        _cache[key] = nc

    from concourse.bass_utils import run_bass_kernel_spmd

    res = run_bass_kernel_spmd(
        nc, in_maps, list(range(NCORES)), trace=bool(PROFILE)
    )
    LAST_EXEC_NS = res.exec_time_ns
    LAST_RESULTS = res

    # unpack ---------------------------------------------------------------
    out = np.empty((e, fo), dtype=np.float32)
    for k in range(NCORES):
        o = res.results[k]["outT"]
        i0, i1 = k * ec, min((k + 1) * ec, e)
        n = i1 - i0
        ok = np.empty((ec, fo), dtype=np.float32)
        for t in range(ntiles):
            w = min(TILE, ec - t * TILE)
            c = out_col(t)
            r = (t % 2) * 64
            ok[t * TILE : t * TILE + w] = o[r : r + fo, c : c + w].T
        if perm is None:
            out[i0:i1] = ok[:n]
        else:
            out[perm[i0:i1]] = ok[:n]
    return out


if __name__ == "__main__":
    # small self-test with synthetic inputs (E scaled down)
    rng = np.random.default_rng(0)
    E, FX, FU, H, FO, B = 40960, 64, 64, 256, 64, 512
    src = rng.standard_normal((E, FX), dtype=np.float32)
    dest = rng.standard_normal((E, FX), dtype=np.float32)
    u = rng.standard_normal((B, FU), dtype=np.float32)
    batch = np.sort(rng.integers(0, B, E)).astype(np.int64)
    W1 = (rng.standard_normal((2 * FX + FU, H), dtype=np.float32) / np.sqrt(2 * FX + FU))
    b1 = np.zeros(H, np.float32)
    W2 = rng.standard_normal((H, FO), dtype=np.float32) / np.sqrt(H)
    b2 = np.zeros(FO, np.float32)
    got = kernel(src=src, dest=dest, edge_attr=src, u=u, batch=batch,
                 W1=W1, b1=b1, W2=W2, b2=b2)
    x = np.concatenate([dest, src, u[batch]], axis=1)
    hh = np.maximum(x @ W1 + b1, 0.0)
    want = hh @ W2 + b2
    rel = np.linalg.norm(got - want) / np.linalg.norm(want)
    print("rel err:", rel)
